# revision 4
# baseline (speedup 1.0000x reference)
"""Trainium2 Bass kernel v4 for nn_Enhanced_transformer (dense transformer).

Data-parallel: one batch element per core (B=8 on 8 cores), channel-major
layout [chan-part, token-free] everywhere, zero runtime transposes.

v4 vs v3: DMA batching. Each dma_start costs ~1.3us of sequencer issue
time, so per-chunk tile loads/stores collapse into ONE big DMA using 3D
access patterns ([row 128] x [chan-tile KP] x [col CH]); bias vectors
gather into [128, KP] tiles with one DMA. Loads issue on SP, stores on ACT.

  h^T = LN1(x)^T          stats via ones-matmul partition reduction (bf16)
  x_q  = h_r' @ qk_w      f32r   [tok-part, q-free]
  energy = xq' @ xq       f32r   PSUM-accumulated over all chunks
  A1   = energy' @ t1_w   f32r   + t1_b -> gelu -> f32r
  att2 = t2_w' @ A1g      f32r   + t2_b -> softmax -> bf16 (SBUF resident)
  x_v^T = v_w' @ h_r      f32r   + v_b -> bf16 spill to DRAM
  t_out^T = att' @ x_v^T  bf16 ;  x1^T = bf16(t_out^T + x^T)
  h2^T = LN2(x1)^T -> bf16 ;  m = gelu(m1' @ h2 + b) bf16
  out = m2' @ m + b + x1  fp32
"""

import numpy as np

import concourse.bass as bass
import concourse.tile as tile
from concourse import bacc, mybir
from concourse import bass_utils

F32 = mybir.dt.float32
F32R = mybir.dt.float32r
BF16 = mybir.dt.bfloat16
AF = mybir.ActivationFunctionType
ALU = mybir.AluOpType
AX = mybir.AxisListType

B, N, P = 8, 4096, 1024
P4 = P // 4          # 256
EPS = 1e-5
CH = 512             # token chunk
NCH = N // CH        # 8
KP = P // 128        # 8 channel tiles
KQ = P4 // 128       # 2


def _tiled_rows_ap(dram_ap, rows, ntile, cols, col_off=0):
    """3D AP over a [rows*ntile, >=col_off+cols] dram tensor:
    iteration (r in rows) x (t in ntile) x (j in cols), element
    [t*rows + r, col_off + j]. Matches an SBUF [rows, ntile*cols] tile."""
    base = dram_ap[0:rows, col_off : col_off + cols]
    return bass.AP(
        tensor=base.tensor, offset=base.offset,
        ap=[base.ap[0], [rows * base.ap[0][0], ntile], base.ap[1]],
    )


def _build(apply_ln1_affine: bool, apply_ln2_affine: bool, loop_R: int = 1):
    nc = bacc.Bacc("TRN2", target_bir_lowering=False, debug=False)

    # ---- DRAM I/O ----
    xT_d = nc.dram_tensor("xT", [P, N], F32, kind="ExternalInput").ap()
    qk_wT_d = nc.dram_tensor("qk_wT", [P, P4], F32R, kind="ExternalInput").ap()
    v_wT_d = nc.dram_tensor("v_wT", [P, P], F32R, kind="ExternalInput").ap()
    t1_wT_d = nc.dram_tensor("t1_wT", [P4, P], F32, kind="ExternalInput").ap()
    t2_wT_d = nc.dram_tensor("t2_wT", [P4, P], F32, kind="ExternalInput").ap()
    m1_wT_d = nc.dram_tensor("m1_wT", [P, P], BF16, kind="ExternalInput").ap()
    m2_wT_d = nc.dram_tensor("m2_wT", [P, P], BF16, kind="ExternalInput").ap()
    v_b_d = nc.dram_tensor("v_b", [P], F32, kind="ExternalInput").ap()
    t1_b_d = nc.dram_tensor("t1_b", [P], F32, kind="ExternalInput").ap()
    t2_b_d = nc.dram_tensor("t2_b", [P], F32, kind="ExternalInput").ap()
    m1_b_d = nc.dram_tensor("m1_b", [P], F32, kind="ExternalInput").ap()
    m2_b_d = nc.dram_tensor("m2_b", [P], F32, kind="ExternalInput").ap()
    ln_d = {}
    if apply_ln1_affine:
        ln_d["ln1_g"] = nc.dram_tensor("ln1_g", [P], F32, kind="ExternalInput").ap()
        ln_d["ln1_b"] = nc.dram_tensor("ln1_b", [P], F32, kind="ExternalInput").ap()
    if apply_ln2_affine:
        ln_d["ln2_g"] = nc.dram_tensor("ln2_g", [P], F32, kind="ExternalInput").ap()
        ln_d["ln2_b"] = nc.dram_tensor("ln2_b", [P], F32, kind="ExternalInput").ap()
    outT_d = nc.dram_tensor("outT", [P, N], F32, kind="ExternalOutput").ap()

    def part_bias_tiles(pool, dram_ap, name):
        """[P] dram vector -> [128, KP] SBUF tile via ONE gather DMA;
        returns per-chan-tile [128,1] column APs."""
        bt = pool.tile([128, KP], F32, tag=name, name=name)
        base = dram_ap[0:128]
        src3 = bass.AP(tensor=base.tensor, offset=base.offset,
                       ap=[base.ap[0], [128, KP]])
        nc.scalar.dma_start(bt[:], src3)
        return [bt[:, t : t + 1] for t in range(KP)]

    with tile.TileContext(nc) as tc:
        with (
            tc.tile_pool(name="dram", bufs=1, space="DRAM") as dram_pool,
            tc.tile_pool(name="consts", bufs=1) as consts,
        ):
            xv_sp = dram_pool.tile([P, N], BF16, name="xv_sp")

            ones_f = consts.tile([128, 128], F32, tag="ones_f", name="ones_f")
            nc.vector.memset(ones_f[:], 1.0 / P)
            ones_b = consts.tile([128, 128], BF16, tag="ones_b", name="ones_b")
            nc.vector.tensor_copy(ones_b[:], ones_f[:])
            eps_t = consts.tile([128, 1], F32, tag="eps", name="eps_t")
            nc.vector.memset(eps_t[:], EPS)

            vb_t = part_bias_tiles(consts, v_b_d, "vb")
            t2b_t = part_bias_tiles(consts, t2_b_d, "t2b")
            m1b_t = part_bias_tiles(consts, m1_b_d, "m1b")
            m2b_t = part_bias_tiles(consts, m2_b_d, "m2b")
            ln_t = {}
            if apply_ln1_affine:
                ln_t["g1"] = part_bias_tiles(consts, ln_d["ln1_g"], "g1")
                ln_t["b1"] = part_bias_tiles(consts, ln_d["ln1_b"], "b1")
            if apply_ln2_affine:
                ln_t["g2"] = part_bias_tiles(consts, ln_d["ln2_g"], "g2")
                ln_t["b2"] = part_bias_tiles(consts, ln_d["ln2_b"], "b2")

            def ln_stats(psP, pP, x_tiles, sq_tiles, tag):
                """x/sq tiles: KP slice-APs [128,CH] bf16. -> (mu_b, rho_b)
                [128,CH] fp32, broadcast to all partitions."""
                ps_s = psP.tile([128, CH], F32, tag=f"{tag}_s", name=f"{tag}_s")
                ps_q = psP.tile([128, CH], F32, tag=f"{tag}_q", name=f"{tag}_q")
                for p in range(KP):
                    nc.tensor.matmul(
                        ps_s[:], ones_b[:], x_tiles[p],
                        start=(p == 0), stop=(p == KP - 1),
                    )
                for p in range(KP):
                    nc.tensor.matmul(
                        ps_q[:], ones_b[:], sq_tiles[p],
                        start=(p == 0), stop=(p == KP - 1),
                    )
                mu_b = pP.tile([128, CH], F32, tag=f"{tag}_mu", name=f"{tag}_mu",
                               bufs=2)
                nc.vector.tensor_copy(mu_b[:], ps_s[:])
                var = pP.tile([128, CH], F32, tag=f"{tag}_var", name=f"{tag}_var",
                              bufs=1)
                nc.vector.tensor_mul(var[:], mu_b[:], mu_b[:])
                nc.vector.tensor_tensor(var[:], ps_q[:], var[:], ALU.subtract)
                nc.scalar.activation(var[:], var[:], AF.Sqrt, bias=eps_t[:])
                rho_b = pP.tile([128, CH], F32, tag=f"{tag}_rho", name=f"{tag}_rho",
                                bufs=2)
                nc.vector.reciprocal(rho_b[:], var[:])
                return mu_b, rho_b

            # Optional hardware repeat-loop for timing (test.py only).
            from contextlib import ExitStack as _ES
            _loop_ctx = _ES()
            if loop_R > 1:
                _loop_ctx.enter_context(tc.For_i(0, loop_R, 1))

            with tc.tile_pool(name="wTop", bufs=1) as wTop:
                # m1/m2 weights + att resident across the iteration. DMAs for
                # m1/m2 are issued inside phase A (after chunk 1/2 loads) so
                # they don't delay the first x chunk.
                m1_w = wTop.tile([128, KP * P], BF16, tag="m1w", name="m1w")
                m2_w = wTop.tile([128, KP * P], BF16, tag="m2w", name="m2w")
                att_t = [
                    wTop.tile([128, P], BF16, tag=f"att{p}", name=f"att{p}")
                    for p in range(KP)
                ]

                def m1_sl(p, j):  # lhsT [128,128] for m1: chan-tile p, out j
                    return m1_w[:, p * P + j * 128 : p * P + (j + 1) * 128]

                def m2_sl(j, o):
                    return m2_w[:, j * P + o * 128 : j * P + (o + 1) * 128]

                # ============ PHASE A: LN1, x_q, energy, x_v ============
                es_ps = _ES()
                es_sb = _ES()
                psE = es_ps.enter_context(
                    tc.tile_pool(name="psE", bufs=1, space="PSUM"))
                wT12 = es_sb.enter_context(tc.tile_pool(name="wT12", bufs=1))
                if True:
                    e_ps = [psE.tile([128, P4], F32, tag=f"e{i}", name=f"e_ps{i}")
                            for i in range(KQ)]
                    t1_w = wT12.tile([128, KQ * P], F32, tag="t1w", name="t1w")
                    t2_w = wT12.tile([128, KQ * P], F32, tag="t2w", name="t2w")
                    with (
                        tc.tile_pool(name="wA", bufs=1) as wA,
                        tc.tile_pool(name="pA", bufs=1) as pA,
                        tc.tile_pool(name="psA", bufs=1, space="PSUM") as psA,
                    ):
                        v_w = wA.tile([128, KP * P], F32R, tag="vw", name="vw")
                        qk_w = wA.tile([128, KP * P4], F32R, tag="qkw", name="qkw")

                        def make_chunk_inputs(c):
                            """Create xt tile, issue its DMA, emit xr/sq ops.
                            Called one chunk ahead so ACT does chunk c+1's
                            squares before chunk c's xv-bias ops."""
                            xt = pA.tile([128, KP * CH], F32, tag="xt",
                                         name="xt", bufs=2)
                            if c == 0:
                                h_kp = KP // 2
                                nc.sync.dma_start(
                                    xt[:, : h_kp * CH],
                                    _tiled_rows_ap(xT_d, 128, h_kp, CH, 0))
                                nc.sync.dma_start(
                                    xt[:, h_kp * CH :],
                                    bass.AP(
                                        tensor=xT_d.tensor,
                                        offset=xT_d[h_kp * 128, 0].offset,
                                        ap=_tiled_rows_ap(
                                            xT_d, 128, h_kp, CH, 0).ap,
                                    ))
                            else:
                                nc.sync.dma_start(
                                    xt[:],
                                    _tiled_rows_ap(xT_d, 128, KP, CH, c * CH))
                            xts = [xt[:, p * CH : (p + 1) * CH]
                                   for p in range(KP)]
                            xr, sq = [], []
                            for p in range(KP):
                                r = pA.tile([128, CH], BF16, tag=f"xr{p}",
                                            name=f"xr{p}", bufs=1)
                                nc.gpsimd.tensor_copy(r[:], xts[p])
                                xr.append(r[:])
                                s = pA.tile([128, CH], BF16, tag=f"sq{p}",
                                            name=f"sq{p}", bufs=1)
                                nc.scalar.activation(s[:], xts[p], AF.Square)
                                sq.append(s[:])
                            return xts, xr, sq

                        nxt = make_chunk_inputs(0)
                        for c in range(NCH):
                            cs = slice(c * CH, (c + 1) * CH)
                            xts, xr, sq = nxt
                            if c == 0:
                                nc.sync.dma_start(
                                    qk_w[:], _tiled_rows_ap(qk_wT_d, 128, KP, P4))
                                nc.sync.dma_start(
                                    v_w[:], _tiled_rows_ap(v_wT_d, 128, KP, P))
                            elif c == 1:
                                nc.sync.dma_start(
                                    m1_w[:], _tiled_rows_ap(m1_wT_d, 128, KP, P))
                            elif c == 2:
                                nc.sync.dma_start(
                                    m2_w[:], _tiled_rows_ap(m2_wT_d, 128, KP, P))
                            elif c == 3:
                                nc.sync.dma_start(
                                    t1_w[:], _tiled_rows_ap(t1_wT_d, 128, KQ, P))
                                nc.sync.dma_start(
                                    t2_w[:], _tiled_rows_ap(t2_wT_d, 128, KQ, P))

                            mu_b, rho_b = ln_stats(psA, pA, xr, sq, "st1")

                            h_r = []
                            for p in range(KP):
                                hr = pA.tile([128, CH], F32R, tag=f"hr{p}",
                                             name=f"hr{p}", bufs=2)
                                nc.vector.tensor_tensor(hr[:], xts[p], mu_b[:],
                                                        ALU.subtract)
                                nc.vector.tensor_mul(hr[:], hr[:], rho_b[:])
                                if apply_ln1_affine:
                                    nc.scalar.activation(
                                        hr[:], hr[:], AF.Identity,
                                        bias=ln_t["b1"][p],
                                        scale=ln_t["g1"][p],
                                    )
                                h_r.append(hr)

                            # x_q chunk + energy accumulation (f32r)
                            for ns in range(CH // 128):
                                ps = psA.tile([128, P4], F32, tag="xq",
                                              name="xq_ps", bufs=2)
                                for p in range(KP):
                                    nc.tensor.matmul(
                                        ps[:],
                                        h_r[p][:, ns * 128 : (ns + 1) * 128],
                                        qk_w[:, p * P4 : (p + 1) * P4],
                                        start=(p == 0), stop=(p == KP - 1),
                                    )
                                xq = pA.tile([128, P4], F32R, tag="xqs",
                                             name="xqs", bufs=2)
                                nc.vector.tensor_copy(xq[:], ps[:])
                                first = c == 0 and ns == 0
                                last = c == NCH - 1 and ns == CH // 128 - 1
                                for qh in range(KQ):
                                    nc.tensor.matmul(
                                        e_ps[qh][:],
                                        xq[:, qh * 128 : (qh + 1) * 128],
                                        xq[:],
                                        start=first, stop=last,
                                        skip_group_check=True,
                                    )

                            # prefetch next chunk: x DMA + xr/sq emission so
                            # ACT squares(c+1) precede xv-bias(c) in its FIFO
                            if c + 1 < NCH:
                                nxt = make_chunk_inputs(c + 1)

                            # x_v^T chunk (f32r GEMM) -> bf16, one spill DMA
                            xv_bg = pA.tile([128, KP * CH], BF16, tag="xvs",
                                            name="xvs", bufs=1)
                            for o in range(KP):
                                ps = psA.tile([128, CH], F32, tag="xv",
                                              name="xv_ps", bufs=2)
                                for p in range(KP):
                                    nc.tensor.matmul(
                                        ps[:],
                                        v_w[:, p * P + o * 128 : p * P + (o + 1) * 128],
                                        h_r[p][:],
                                        start=(p == 0), stop=(p == KP - 1),
                                    )
                                nc.scalar.activation(
                                    xv_bg[:, o * CH : (o + 1) * CH], ps[:],
                                    AF.Identity, bias=vb_t[o]
                                )
                            nc.scalar.dma_start(
                                _tiled_rows_ap(xv_sp, 128, KP, CH, c * CH),
                                xv_bg[:],
                            )

                    # ============ PHASE B: logits + softmax (f32r) ============
                    pCx_pool = es_sb.enter_context(
                        tc.tile_pool(name="pCx", bufs=1))
                    with (
                        tc.tile_pool(name="wB", bufs=1) as wB,
                        tc.tile_pool(name="pB", bufs=1) as pB,
                        tc.tile_pool(name="psB", bufs=1, space="PSUM") as psB,
                    ):
                        t1b_bc = wB.tile([128, P], F32, tag="t1b_bc",
                                         name="t1b_bc")
                        t1b_src = bass.AP(
                            tensor=t1_b_d.tensor, offset=t1_b_d.offset,
                            ap=[[0, 128], *t1_b_d.ap],
                        )
                        nc.sync.dma_start(t1b_bc[:], t1b_src)

                        energy_sb = []
                        for qh in range(KQ):
                            e = pB.tile([128, P4], F32, tag=f"esb{qh}",
                                        name=f"esb{qh}")
                            nc.vector.tensor_copy(e[:], e_ps[qh][:])
                            energy_sb.append(e)

                        # A1[k, o] = sum_q energy[q,k] t1_wT[q,o]; +t1_b; gelu
                        a1g = []
                        for bh in range(KQ):
                            a1 = pB.tile([128, P], F32, tag=f"a1_{bh}",
                                         name=f"a1_{bh}")
                            for oc in range(P // 512):
                                ps = psB.tile([128, 512], F32, tag="a1",
                                              name="a1_ps", bufs=2)
                                for qh in range(KQ):
                                    nc.tensor.matmul(
                                        ps[:],
                                        energy_sb[qh][:, bh * 128 : (bh + 1) * 128],
                                        t1_w[:, qh * P + oc * 512 : qh * P + (oc + 1) * 512],
                                        start=(qh == 0), stop=(qh == KQ - 1),
                                    )
                                nc.vector.tensor_tensor(
                                    a1[:, oc * 512 : (oc + 1) * 512], ps[:],
                                    t1b_bc[:, oc * 512 : (oc + 1) * 512], ALU.add,
                                )
                            ag = pB.tile([128, P], F32, tag=f"a1g{bh}",
                                         name=f"a1g{bh}")
                            nc.scalar.activation(ag[:], a1[:], AF.Gelu)
                            a1g.append(ag)

                        # att2 + softmax -> att_t (bf16, resident)
                        for o in range(KP):
                            att2 = pB.tile([128, P], F32, tag="att2", name="att2",
                                           bufs=2)
                            for kc in range(P // 512):
                                ps = psB.tile([128, 512], F32, tag="a2",
                                              name="a2_ps", bufs=2)
                                for ph in range(KQ):
                                    nc.tensor.matmul(
                                        ps[:],
                                        t2_w[:, ph * P + o * 128 : ph * P + (o + 1) * 128],
                                        a1g[ph][:, kc * 512 : (kc + 1) * 512],
                                        start=(ph == 0), stop=(ph == KQ - 1),
                                    )
                                nc.scalar.activation(
                                    att2[:, kc * 512 : (kc + 1) * 512], ps[:],
                                    AF.Identity, bias=t2b_t[o],
                                )
                            negmax = pB.tile([128, 1], F32, tag="negmax",
                                             name="negmax", bufs=2)
                            nc.vector.tensor_reduce(
                                negmax[:], att2[:], axis=AX.X, op=ALU.max,
                                negate=True
                            )
                            esum = pB.tile([128, 1], F32, tag="esum", name="esum",
                                           bufs=2)
                            expv = pB.tile([128, P], F32, tag="expv", name="expv",
                                           bufs=2)
                            nc.scalar.activation(
                                expv[:], att2[:], AF.Exp, bias=negmax[:],
                                accum_out=esum[:],
                            )
                            rec = pB.tile([128, 1], F32, tag="rec", name="rec",
                                          bufs=2)
                            nc.vector.reciprocal(rec[:], esum[:])
                            nc.vector.tensor_scalar_mul(att_t[o][:], expv[:],
                                                        rec[:])

                es_ps.close()

                # ============ PHASE C: t_out, x1, LN2, MLP, out ============
                with (
                    tc.tile_pool(name="pC", bufs=1) as pC,
                    tc.tile_pool(name="psC", bufs=1, space="PSUM") as psC,
                ):
                    for c in range(NCH):
                        xt = pCx_pool.tile([128, KP * CH], F32, tag="xt",
                                           name="xt", bufs=2)
                        nc.sync.dma_start(
                            xt[:], _tiled_rows_ap(xT_d, 128, KP, CH, c * CH))
                        xts = [xt[:, p * CH : (p + 1) * CH] for p in range(KP)]
                        xvl = pCx_pool.tile([128, KP * CH], BF16, tag="xvl",
                                            name="xvl", bufs=2)
                        nc.sync.dma_start(
                            xvl[:], _tiled_rows_ap(xv_sp, 128, KP, CH, c * CH))
                        xvs = [xvl[:, p * CH : (p + 1) * CH] for p in range(KP)]

                        x1f, sq2 = [], []
                        for q in range(KP):
                            ps = psC.tile([128, CH], F32, tag="tout",
                                          name="tout_ps", bufs=2)
                            for p in range(KP):
                                nc.tensor.matmul(
                                    ps[:],
                                    att_t[p][:, q * 128 : (q + 1) * 128],
                                    xvs[p],
                                    start=(p == 0), stop=(p == KP - 1),
                                )
                            x1 = pC.tile([128, CH], BF16, tag=f"x1{q}",
                                         name=f"x1{q}", bufs=2)
                            nc.vector.tensor_tensor(x1[:], ps[:], xts[q],
                                                    ALU.add)
                            s = pC.tile([128, CH], BF16, tag=f"sq2{q}",
                                        name=f"sq2{q}", bufs=2)
                            nc.scalar.activation(s[:], x1[:], AF.Square)
                            x1f.append(x1)
                            sq2.append(s[:])

                        mu2, rho2 = ln_stats(psC, pC,
                                             [t[:] for t in x1f], sq2, "st2")

                        h2 = []
                        for q in range(KP):
                            tmp = pC.tile([128, CH], F32, tag="h2tmp",
                                          name="h2tmp", bufs=2)
                            nc.vector.tensor_tensor(tmp[:], x1f[q][:], mu2[:],
                                                    ALU.subtract)
                            hb = pC.tile([128, CH], BF16, tag=f"h2{q}",
                                         name=f"h2{q}", bufs=2)
                            nc.vector.tensor_mul(hb[:], tmp[:], rho2[:])
                            if apply_ln2_affine:
                                nc.scalar.activation(
                                    hb[:], hb[:], AF.Identity,
                                    bias=ln_t["b2"][q], scale=ln_t["g2"][q],
                                )
                            h2.append(hb)

                        mg = []
                        for j in range(KP):
                            ps = psC.tile([128, CH], F32, tag="m1", name="m1_ps",
                                          bufs=2)
                            for p in range(KP):
                                nc.tensor.matmul(
                                    ps[:],
                                    m1_sl(p, j),
                                    h2[p][:],
                                    start=(p == 0), stop=(p == KP - 1),
                                )
                            g = pC.tile([128, CH], BF16, tag=f"mg{j}",
                                        name=f"mg{j}", bufs=2)
                            nc.scalar.activation(g[:], ps[:], AF.Gelu,
                                                 bias=m1b_t[j])
                            mg.append(g)

                        mo = pC.tile([128, KP * CH], F32, tag="mo", name="mo",
                                     bufs=1)
                        last = c == NCH - 1
                        for o in range(KP):
                            ps = psC.tile([128, CH], F32, tag="m2", name="m2_ps",
                                          bufs=2)
                            for j in range(KP):
                                nc.tensor.matmul(
                                    ps[:],
                                    m2_sl(j, o),
                                    mg[j][:],
                                    start=(j == 0), stop=(j == KP - 1),
                                )
                            nc.vector.scalar_tensor_tensor(
                                mo[:, o * CH : (o + 1) * CH], ps[:], m2b_t[o],
                                x1f[o][:],
                                op0=ALU.add, op1=ALU.add,
                            )
                            if last and o == KP // 2 - 1:
                                nc.scalar.dma_start(
                                    _tiled_rows_ap(outT_d, 128, KP // 2, CH,
                                                   c * CH),
                                    mo[:, : KP // 2 * CH])
                        if last:
                            half = bass.AP(
                                tensor=outT_d.tensor,
                                offset=outT_d[KP // 2 * 128, c * CH].offset,
                                ap=_tiled_rows_ap(outT_d, 128, KP // 2, CH,
                                                  c * CH).ap,
                            )
                            nc.scalar.dma_start(half, mo[:, KP // 2 * CH :])
                        else:
                            nc.scalar.dma_start(
                                _tiled_rows_ap(outT_d, 128, KP, CH, c * CH),
                                mo[:])

                es_sb.close()

            _loop_ctx.close()

    nc.compile()
    return nc


_CACHE = {}


def _get_nc(apply_ln1_affine, apply_ln2_affine, loop_R=1):
    key = (apply_ln1_affine, apply_ln2_affine, loop_R)
    if key not in _CACHE:
        _CACHE[key] = _build(apply_ln1_affine, apply_ln2_affine, loop_R)
    return _CACHE[key]


def _round_f32r(x):
    """Round fp32 -> tf32-like (10 explicit mantissa bits, RNE)."""
    u = np.ascontiguousarray(x, np.float32).view(np.uint32)
    shift = 13
    bias = np.uint32((1 << (shift - 1)) - 1)
    lsb = (u >> np.uint32(shift)) & np.uint32(1)
    u2 = (u + bias + lsb) & np.uint32(~((1 << shift) - 1) & 0xFFFFFFFF)
    return u2.view(np.float32)


def _to_bf16(x):
    import ml_dtypes

    return np.ascontiguousarray(np.asarray(x, np.float32)).astype(
        ml_dtypes.bfloat16
    )


def kernel(**inputs):
    return _kernel_impl(inputs, loop_R=1)


def _kernel_impl(inputs, loop_R=1, trace=False, tmpdir=None):
    x = np.ascontiguousarray(np.asarray(inputs["x"], np.float32))
    assert x.shape == (B, N, P), x.shape

    ln1_g = np.asarray(inputs["ln1_g"], np.float32)
    ln1_b = np.asarray(inputs["ln1_b"], np.float32)
    ln2_g = np.asarray(inputs["ln2_g"], np.float32)
    ln2_b = np.asarray(inputs["ln2_b"], np.float32)
    aff1 = not (np.all(ln1_g == 1.0) and np.all(ln1_b == 0.0))
    aff2 = not (np.all(ln2_g == 1.0) and np.all(ln2_b == 0.0))

    nc = _get_nc(aff1, aff2, loop_R)

    base = {
        "qk_wT": _round_f32r(np.asarray(inputs["qk_w"], np.float32).T),
        "v_wT": _round_f32r(np.asarray(inputs["v_w"], np.float32).T),
        "t1_wT": np.ascontiguousarray(np.asarray(inputs["t1_w"], np.float32).T),
        "t2_wT": np.ascontiguousarray(np.asarray(inputs["t2_w"], np.float32).T),
        "m1_wT": _to_bf16(np.asarray(inputs["m1_w"], np.float32).T),
        "m2_wT": _to_bf16(np.asarray(inputs["m2_w"], np.float32).T),
        "v_b": np.ascontiguousarray(np.asarray(inputs["v_b"], np.float32)),
        "t1_b": np.ascontiguousarray(np.asarray(inputs["t1_b"], np.float32)),
        "t2_b": np.ascontiguousarray(np.asarray(inputs["t2_b"], np.float32)),
        "m1_b": np.ascontiguousarray(np.asarray(inputs["m1_b"], np.float32)),
        "m2_b": np.ascontiguousarray(np.asarray(inputs["m2_b"], np.float32)),
    }
    if aff1:
        base["ln1_g"] = np.ascontiguousarray(ln1_g)
        base["ln1_b"] = np.ascontiguousarray(ln1_b)
    if aff2:
        base["ln2_g"] = np.ascontiguousarray(ln2_g)
        base["ln2_b"] = np.ascontiguousarray(ln2_b)

    in_maps = []
    for b in range(B):
        m = dict(base)
        m["xT"] = np.ascontiguousarray(x[b].T)
        in_maps.append(m)

    res = bass_utils.run_bass_kernel_spmd(
        nc, in_maps, core_ids=list(range(B)), trace=trace, tmpdir=tmpdir
    )
    out = np.empty((B, N, P), np.float32)
    for b in range(B):
        out[b] = res.results[b]["outT"].T
    if trace:
        return out, res
    return out


if __name__ == "__main__":
    import sys
    import time

    sys.path.insert(0, "/root/problem")
    import reference as refmod

    inputs = {k: np.asarray(v) for k, v in refmod.setup_inputs().items()}
    t0 = time.time()
    got = kernel(**inputs)
    print(f"kernel() took {time.time() - t0:.1f}s (incl compile)")
    exp = np.asarray(refmod.reference(**inputs))
    err = np.abs(got - exp)
    scale = np.abs(exp).max()
    l2 = np.linalg.norm(got - exp) / np.linalg.norm(exp)
    print(f"absmax={err.max():.3e} scale-rel={err.max() / scale:.3e} L2rel={l2:.3e}")
